# revision 19
# baseline (speedup 1.0000x reference)
"""ODE-RNN encoder (GRU-ODE scan) Trainium2 Bass kernel.

Strategy (data-parallel over trajectories):
  - 4096 trajectories sharded 512/core over 8 NeuronCores; all weights
    replicated. The T=128 time scan runs locally per core, no cross-core
    communication. Host gathers the per-core z0 outputs at the end.
  - On-chip layout is feature-on-partition, batch-on-free-dim, bf16
    everywhere except PSUM accumulation. Each core's 512-batch is split
    into 2 anti-phase chunks of 256; the emission is an explicit
    software-pipelined window schedule (one window = one step of chunk A
    + tail of chunk B's previous step + head of chunk B's current step)
    so the PE queue always has the other chunk's independent matmuls to
    chew on while one chunk's serial chain hops engines.
  - 11 matmuls per chunk-step: ode1, g1x_u, g1x_r, ns1x, ode2, g1s_u,
    g1s_r, rg2, ug2, ns1s, ns2. The ODE second layer is pre-scaled by dt
    on the host (per-distinct-dt weight copies) so the Euler update is a
    single DVE add. Gate-1 state matmuls read the post-ODE state written
    in place by that add.
  - Engine balance per chunk-step: PE 11 matmuls, ACT 6 activations
    (tanh_ode, tanh_r, tanh_u, sig_r, sig_v, tanh_ns), DVE 8 (yode, ryc,
    gm, gt1, uy, abs(=abs_max), gtq_top, add_top), Pool 3 SBUF-only tail
    blends (qb, gtq_bot, add_bot; GPSIMD has no PSUM port). The mask
    duplicate m2 is DMAed straight from DRAM on the sync queue.
  - PSUM: 4 banks per chunk (A1=[ode hidden|ode out], A2=[gate1 u|r],
    B1=[gate2 u|r], B2=[ns hidden|ns out]); start=True zeroes a whole
    bank, cross-step safety is by queue order (see schedule comments).

kernel(**inputs) takes the full unsharded numpy inputs (as produced by the
reference setup) and returns (z0_mu, z0_std), each (1, 4096, 64) float32.
"""

import os
import sys

import numpy as np

N_TRAJ = 4096
T = 128
LAT = 64
NDATA = 64
INP = 2 * NDATA
NGRU = 100
NODE = 100
TZ = 100
NCORES = 8
B = N_TRAJ // NCORES          # 512 per core
CH = 2                        # chunks per core
BC = B // CH                  # 256 batch per chunk

_cache = {}


def _build(dts, use_bias):
    import concourse.bass as bass
    import concourse.tile as tile
    from concourse import bacc, mybir
    from concourse.tile import add_dep_helper

    uniq = list(dict.fromkeys(dts))
    dt_idx = [uniq.index(d) for d in dts]
    n_dt = len(uniq)

    f32 = mybir.dt.float32
    bf16 = mybir.dt.bfloat16
    ACT = mybir.ActivationFunctionType
    ALU = mybir.AluOpType

    nc = bacc.Bacc("TRN2", target_bir_lowering=False, debug=False,
                   num_devices=NCORES)

    # ---- DRAM I/O ----
    xT_d = nc.dram_tensor("xT", [T, INP, B], bf16, kind="ExternalInput")
    mT_d = nc.dram_tensor("mT", [T, INP, B], bf16, kind="ExternalInput")
    wug1_d = nc.dram_tensor("wug1", [2 * LAT + INP, NGRU], bf16, kind="ExternalInput")
    wrg1_d = nc.dram_tensor("wrg1", [2 * LAT + INP, NGRU], bf16, kind="ExternalInput")
    wns1_d = nc.dram_tensor("wns1", [2 * LAT + INP, NGRU], bf16, kind="ExternalInput")
    wug2_d = nc.dram_tensor("wug2nd", [NGRU, 2 * LAT], bf16, kind="ExternalInput")
    wrg2_d = nc.dram_tensor("wrg2d", [NGRU, 2 * LAT], bf16, kind="ExternalInput")
    wns2_d = nc.dram_tensor("wns2", [NGRU, 2 * LAT], bf16, kind="ExternalInput")
    wode1_d = nc.dram_tensor("wode1", [LAT, NODE], bf16, kind="ExternalInput")
    wode2_d = nc.dram_tensor("wode2dt", [n_dt, NODE, LAT], bf16, kind="ExternalInput")
    wtz1_d = nc.dram_tensor("wtz1", [2 * LAT, TZ], bf16, kind="ExternalInput")
    wtz2_d = nc.dram_tensor("wtz2", [TZ, 2 * LAT], bf16, kind="ExternalInput")
    if use_bias:
        bode1_d = nc.dram_tensor("bode1", [NODE, 1], f32, kind="ExternalInput")
        bug1_d = nc.dram_tensor("bug1", [NGRU, 1], f32, kind="ExternalInput")
        brg1_d = nc.dram_tensor("brg1", [NGRU, 1], f32, kind="ExternalInput")
        bns1_d = nc.dram_tensor("bns1", [NGRU, 1], f32, kind="ExternalInput")
        bns2b_d = nc.dram_tensor("bns2b", [LAT, 1], f32, kind="ExternalInput")
        btz1_d = nc.dram_tensor("btz1", [TZ, 1], f32, kind="ExternalInput")
        btz2t_d = nc.dram_tensor("btz2t", [LAT, 1], f32, kind="ExternalInput")
        btz2b_d = nc.dram_tensor("btz2b", [LAT, 1], f32, kind="ExternalInput")
        # row-vector biases (K=1 matmul accumulate): [1, M]
        bug2_d = nc.dram_tensor("bug2ndr", [1, 2 * LAT], bf16, kind="ExternalInput")
        brg2_d = nc.dram_tensor("brg2dr", [1, 2 * LAT], bf16, kind="ExternalInput")
        bns2t_d = nc.dram_tensor("bns2tr", [1, LAT], bf16, kind="ExternalInput")
        bode2_d = nc.dram_tensor("bode2rdt", [n_dt, 1, LAT], bf16, kind="ExternalInput")
        ones_d = nc.dram_tensor("ones1", [1, BC], bf16, kind="ExternalInput")
    zeros_d = nc.dram_tensor("zeros0", [2 * LAT, B], bf16, kind="ExternalInput")
    zout_d = nc.dram_tensor("zout", [2 * LAT, B], f32, kind="ExternalOutput")

    with tile.TileContext(nc) as tc:
        with (
            tc.tile_pool(name="const", bufs=1) as cpool,
            tc.tile_pool(name="state", bufs=1) as spool,
            tc.tile_pool(name="xin", bufs=3) as xpool,
            tc.tile_pool(name="mdup", bufs=3) as mpool,
            tc.tile_pool(name="tmp0", bufs=2) as tpool0,
            tc.tile_pool(name="tmp1", bufs=2) as tpool1,
            tc.tile_pool(name="bkA10", bufs=1, space="PSUM") as bA10,
            tc.tile_pool(name="bkA20", bufs=1, space="PSUM") as bA20,
            tc.tile_pool(name="bkB10", bufs=1, space="PSUM") as bB10,
            tc.tile_pool(name="bkB20", bufs=1, space="PSUM") as bB20,
            tc.tile_pool(name="bkA11", bufs=1, space="PSUM") as bA11,
            tc.tile_pool(name="bkA21", bufs=1, space="PSUM") as bA21,
            tc.tile_pool(name="bkB11", bufs=1, space="PSUM") as bB11,
            tc.tile_pool(name="bkB21", bufs=1, space="PSUM") as bB21,
        ):
            tpool = [tpool0, tpool1]
            bA1p = [bA10, bA11]
            bA2p = [bA20, bA21]
            bB1p = [bB10, bB11]
            bB2p = [bB20, bB21]

            # ---- load constants ----
            def cload(shape, src_ap, tag, dt_=None):
                t = cpool.tile(shape, dt_ or bf16, tag=tag, name=tag)
                nc.sync.dma_start(t[:, :], src_ap)
                return t

            wug1a = cload([INP, NGRU], wug1_d[0:INP, :], "wug1a")
            wug1b = cload([INP, NGRU], wug1_d[INP:2 * LAT + INP, :], "wug1b")
            wrg1a = cload([INP, NGRU], wrg1_d[0:INP, :], "wrg1a")
            wrg1b = cload([INP, NGRU], wrg1_d[INP:2 * LAT + INP, :], "wrg1b")
            wns1a = cload([INP, NGRU], wns1_d[0:INP, :], "wns1a")
            wns1b = cload([INP, NGRU], wns1_d[INP:2 * LAT + INP, :], "wns1b")
            # rows 0:128 of w*1 multiply [y;s] (=128 rows), rows 128:256
            # multiply x (=128 rows); INP == 2*LAT == 128 here.
            wug2 = cload([NGRU, 2 * LAT], wug2_d[:, :], "wug2")
            wrg2 = cload([NGRU, 2 * LAT], wrg2_d[:, :], "wrg2")
            wns2 = cload([NGRU, 2 * LAT], wns2_d[:, :], "wns2")
            wode1 = cload([LAT, NODE], wode1_d[:, :], "wode1")
            wode2 = [cload([NODE, LAT], wode2_d[i], f"wode2_{i}")
                     for i in range(n_dt)]
            wtz1 = cload([2 * LAT, TZ], wtz1_d[:, :], "wtz1")
            wtz2 = cload([TZ, 2 * LAT], wtz2_d[:, :], "wtz2")
            if use_bias:
                bode1 = cload([NODE, 1], bode1_d[:, :], "bode1", f32)
                bug1 = cload([NGRU, 1], bug1_d[:, :], "bug1", f32)
                brg1 = cload([NGRU, 1], brg1_d[:, :], "brg1", f32)
                bns1 = cload([NGRU, 1], bns1_d[:, :], "bns1", f32)
                btz1 = cload([TZ, 1], btz1_d[:, :], "btz1", f32)
                btz2t = cload([LAT, 1], btz2t_d[:, :], "btz2t", f32)
                bns2b = cpool.tile([2 * LAT, 1], f32, tag="bns2b", name="bns2b")
                nc.sync.dma_start(bns2b[LAT:2 * LAT, :], bns2b_d[:, :])
                btz2b = cpool.tile([2 * LAT, 1], f32, tag="btz2b", name="btz2b")
                nc.sync.dma_start(btz2b[LAT:2 * LAT, :], btz2b_d[:, :])
                bug2r = cload([1, 2 * LAT], bug2_d[:, :], "bug2r")
                brg2r = cload([1, 2 * LAT], brg2_d[:, :], "brg2r")
                bns2tr = cload([1, LAT], bns2t_d[:, :], "bns2tr")
                bode2r = [cload([1, LAT], bode2_d[i], f"bode2r_{i}")
                          for i in range(n_dt)]
                ones = cpool.tile([1, BC], bf16, tag="ones", name="ones")
                nc.sync.dma_start(ones[:, :], ones_d[:, :])

            def b_act(t):
                return t[:, :] if use_bias else 0.0

            # ---- state tiles (ping-pong per chunk) ----
            S = [[spool.tile([2 * LAT, BC], bf16, tag=f"s{c}_{p}",
                             name=f"s{c}_{p}")
                  for p in range(2)] for c in range(CH)]
            for c in range(CH):
                nc.sync.dma_start(S[c][0][:, :],
                                  zeros_d[:, c * BC:(c + 1) * BC])

            # ---- PSUM banks: 4 per chunk, 8 total (each [128, 2*BC] f32
            # = 2 KiB/partition = exactly one bank) ----
            bankA1 = [bA1p[c].tile([128, 2 * BC], f32, tag="bA1",
                                   name=f"bA1_{c}") for c in range(CH)]
            bankA2 = [bA2p[c].tile([128, 2 * BC], f32, tag="bA2",
                                   name=f"bA2_{c}") for c in range(CH)]
            bankB1 = [bB1p[c].tile([128, 2 * BC], f32, tag="bB1",
                                   name=f"bB1_{c}") for c in range(CH)]
            bankB2 = [bB2p[c].tile([128, 2 * BC], f32, tag="bB2",
                                   name=f"bB2_{c}") for c in range(CH)]

            ctxs = [[None] * T for _ in range(CH)]

            def get_ctx(c, t):
                if ctxs[c][t] is None:
                    ctxs[c][t] = dict(
                        cs=slice(c * BC, (c + 1) * BC),
                        Sc=S[c][t % 2], Sn=S[c][(t + 1) % 2],
                        tp=tpool[c], t=t,
                        A1=bankA1[c], A2=bankA2[c],
                        B1=bankB1[c], B2=bankB2[c])
                return ctxs[c][t]

            # ---------------- stages ----------------
            NAME2STAGE = {}

            def rec(tag, bi):
                try:
                    NAME2STAGE[bi.ins.name] = tag
                except Exception:
                    pass
                return bi

            def s_ode1(c, d, xt, m2):
                d['oh'] = d['A1'][0:NODE, 0:BC]
                rec('mm_ode1', nc.tensor.matmul(d['oh'], wode1[:, :], d['Sc'][0:LAT, :],
                                 start=True, stop=False))

            def s_g1x_u(c, d, xt, m2):
                d['g1'] = d['A2'][0:NGRU, 0:2 * BC]
                rec('mm_g1x_u', nc.tensor.matmul(d['g1'][:, 0:BC], wug1b[:, :],
                                 xt[:, d['cs']], start=True, stop=False))

            def s_g1x_r(c, d, xt, m2):
                rec('mm_g1x_r', nc.tensor.matmul(d['g1'][:, BC:2 * BC], wrg1b[:, :],
                                 xt[:, d['cs']], start=False, stop=False,
                                 skip_group_check=True))

            def s_tanh_ode(c, d, xt, m2):
                d['h_ode'] = d['tp'].tile([NODE, BC], bf16, tag="h_ode",
                                          name=f"ho{c}")
                nc.scalar.activation(d['h_ode'][:, :], d['oh'], ACT.Tanh,
                                     bias=b_act(bode1) if use_bias else 0.0)

            def s_ns1x(c, d, xt, m2):
                d['n1'] = d['B2'][0:NGRU, 0:BC]
                mm = rec('mm_ns1x', nc.tensor.matmul(d['n1'], wns1b[:, :],
                                      xt[:, d['cs']], start=True, stop=False))
                # start=True wipes the whole B2 bank; abs(t-1) reads n2
                # from it on ACT with no downstream tie to this op's
                # queue-order guards, so pin the WAR explicitly.
                prev = ctxs[c][d['t'] - 1] if d['t'] > 0 else None
                if prev is not None and 'abs_i' in prev:
                    add_dep_helper(mm.ins, prev['abs_i'].ins, sync=True,
                                   reason="B2 abs WAR")

            def s_ode2(c, d, xt, m2):
                # wode2 is pre-scaled by dt on host (per distinct dt)
                d['yo'] = d['A1'][0:LAT, BC:2 * BC]
                i = dt_idx[d['t']]
                rec('mm_ode2', nc.tensor.matmul(d['yo'], wode2[i][:, :], d['h_ode'][:, :],
                                 start=False, stop=not use_bias,
                                 skip_group_check=True))
                if use_bias:
                    nc.tensor.matmul(d['yo'], bode2r[i][:, :], ones[:, :],
                                     start=False, stop=True,
                                     skip_group_check=True)

            def s_yode(c, d, xt, m2):
                # in-place Euler: Sc[0:LAT] <- y + (dt*ode2)@h  (STT form
                # for the DVE fast path)
                nc.vector.scalar_tensor_tensor(
                    d['Sc'][0:LAT, :], d['yo'], 0.0, d['Sc'][0:LAT, :],
                    op0=ALU.add, op1=ALU.add)

            def s_g1s_r(c, d, xt, m2):
                rec('mm_g1s_r', nc.tensor.matmul(d['g1'][:, BC:2 * BC], wrg1a[:, :],
                                 d['Sc'][:, :], start=False, stop=False,
                                 skip_group_check=True))

            def s_g1s_u(c, d, xt, m2):
                rec('mm_g1s_u', nc.tensor.matmul(d['g1'][:, 0:BC], wug1a[:, :],
                                 d['Sc'][:, :], start=False, stop=True,
                                 skip_group_check=True))

            def s_tanh_r(c, d, xt, m2):
                d['h_g'] = d['tp'].tile([NGRU, 2 * BC], bf16, tag="h_g",
                                        name=f"hg{c}")
                nc.scalar.activation(d['h_g'][:, BC:2 * BC],
                                     d['g1'][:, BC:2 * BC], ACT.Tanh,
                                     bias=b_act(brg1) if use_bias else 0.0)

            def s_tanh_u(c, d, xt, m2):
                nc.scalar.activation(d['h_g'][:, 0:BC], d['g1'][:, 0:BC],
                                     ACT.Tanh,
                                     bias=b_act(bug1) if use_bias else 0.0)

            def s_rg2(c, d, xt, m2):
                d['g2'] = d['B1'][0:2 * LAT, 0:2 * BC]
                rec('mm_rg2', nc.tensor.matmul(d['g2'][:, BC:2 * BC], wrg2[:, :],
                                 d['h_g'][:, BC:2 * BC],
                                 start=True, stop=False))
                if use_bias:
                    nc.tensor.matmul(d['g2'][:, BC:2 * BC], brg2r[:, :],
                                     ones[:, :], start=False, stop=False,
                                     skip_group_check=True)

            def s_ug2(c, d, xt, m2):
                rec('mm_ug2', nc.tensor.matmul(d['g2'][:, 0:BC], wug2[:, :],
                                 d['h_g'][:, 0:BC],
                                 start=False, stop=not use_bias,
                                 skip_group_check=True))
                if use_bias:
                    nc.tensor.matmul(d['g2'][:, 0:BC], bug2r[:, :],
                                     ones[:, :], start=False, stop=True,
                                     skip_group_check=True)

            def s_sig_ur(c, d, xt, m2):
                # one wide sigmoid over [ (1-u) dup | r dup ] halves
                d['vr'] = d['tp'].tile([2 * LAT, 2 * BC], bf16, tag="vr",
                                       name=f"vr{c}")
                nc.scalar.activation(d['vr'][:, 0:2 * BC],
                                     d['g2'][:, 0:2 * BC], ACT.Sigmoid)

            def s_ryc(c, d, xt, m2):
                d['ryc'] = d['tp'].tile([2 * LAT, BC], bf16, tag="ryc",
                                        name=f"ryc{c}")
                nc.vector.scalar_tensor_tensor(
                    d['ryc'][:, :], d['vr'][:, BC:2 * BC], 0.0,
                    d['Sc'][:, :], op0=ALU.add, op1=ALU.mult)

            def s_gm(c, d, xt, m2):
                d['g'] = d['tp'].tile([2 * LAT, BC], bf16, tag="g",
                                      name=f"g{c}")
                nc.vector.scalar_tensor_tensor(
                    d['g'][:, :], d['vr'][:, 0:BC], 0.0,
                    m2[:, d['cs']], op0=ALU.add, op1=ALU.mult)

            def s_ns1s(c, d, xt, m2):
                rec('mm_ns1s', nc.tensor.matmul(d['n1'], wns1a[:, :],
                                 d['ryc'][:, :], start=False, stop=False,
                                 skip_group_check=True))

            def s_tanh_ns(c, d, xt, m2):
                d['h_n'] = d['tp'].tile([NGRU, BC], bf16, tag="h_n",
                                        name=f"hn{c}")
                nc.scalar.activation(d['h_n'][:, :], d['n1'], ACT.Tanh,
                                     bias=b_act(bns1) if use_bias else 0.0)

            def s_ns2(c, d, xt, m2):
                d['n2'] = d['B2'][0:2 * LAT, BC:2 * BC]
                rec('mm_ns2', nc.tensor.matmul(d['n2'], wns2[:, :], d['h_n'][:, :],
                                 start=False, stop=not use_bias,
                                 skip_group_check=True))
                if use_bias:
                    nc.tensor.matmul(d['n2'][0:LAT, :], bns2tr[:, :],
                                     ones[:, :], start=False, stop=True,
                                     skip_group_check=True)

            # Blends all on DVE as scalar_tensor_tensor (TensorScalarPtr
            # supports the 2x/4x DVE perf modes; plain TensorTensor does
            # not) — and NOTHING on GPSIMD: its elementwise is ~2.3x
            # slower and contends with DVE for the shared SBUF port.
            #   top: Sn_t = (1-g)y_ode + g*ns_t
            #        uyneg = (g-1)*y_ode; gtq = ns_t*g; Sn_t = gtq - uyneg
            #   bot: Sn_b = (1-g)|ns_b| + g*s
            #        sneg = (g-1)*s;     gb = |ns_b|*g; Sn_b = gb - sneg
            def s_uyneg(c, d, xt, m2):
                d['uyneg'] = d['tp'].tile([LAT, BC], bf16, tag="uyneg",
                                          name=f"uyn{c}")
                nc.vector.scalar_tensor_tensor(
                    d['uyneg'][:, :], d['g'][0:LAT, :], 1.0,
                    d['Sc'][0:LAT, :], op0=ALU.subtract, op1=ALU.mult)

            def s_sneg(c, d, xt, m2):
                d['sneg'] = d['tp'].tile([2 * LAT, BC], bf16, tag="sneg",
                                         name=f"sn{c}")
                nc.vector.scalar_tensor_tensor(
                    d['sneg'][LAT:2 * LAT, :], d['g'][LAT:2 * LAT, :], 1.0,
                    d['Sc'][LAT:2 * LAT, :], op0=ALU.subtract, op1=ALU.mult)

            def s_abs(c, d, xt, m2):
                d['absb'] = d['tp'].tile([2 * LAT, BC], bf16, tag="absb",
                                         name=f"ab{c}")
                d['abs_i'] = nc.scalar.activation(
                    d['absb'][LAT:2 * LAT, :], d['n2'][LAT:2 * LAT, :],
                    ACT.Abs,
                    bias=bns2b[LAT:2 * LAT, :] if use_bias else 0.0)

            def s_gtq_top(c, d, xt, m2):
                d['gtq'] = d['tp'].tile([2 * LAT, BC], bf16, tag="gtq",
                                        name=f"gtq{c}")
                nc.vector.scalar_tensor_tensor(
                    d['gtq'][0:LAT, :], d['n2'][0:LAT, :], 0.0,
                    d['g'][0:LAT, :], op0=ALU.add, op1=ALU.mult)

            def s_add_top(c, d, xt, m2):
                nc.vector.scalar_tensor_tensor(
                    d['Sn'][0:LAT, :], d['uyneg'][:, :], -1.0,
                    d['gtq'][0:LAT, :], op0=ALU.mult, op1=ALU.add)

            def s_gb(c, d, xt, m2):
                nc.vector.scalar_tensor_tensor(
                    d['gtq'][LAT:2 * LAT, :], d['absb'][LAT:2 * LAT, :], 0.0,
                    d['g'][LAT:2 * LAT, :], op0=ALU.add, op1=ALU.mult)

            def s_add_bot(c, d, xt, m2):
                nc.vector.scalar_tensor_tensor(
                    d['Sn'][LAT:2 * LAT, :], d['sneg'][LAT:2 * LAT, :], -1.0,
                    d['gtq'][LAT:2 * LAT, :], op0=ALU.mult, op1=ALU.add)

            # ---------------- window schedule ----------------
            # One window = chunk A's full step t, interleaved with chunk
            # B's tail of step t-1 (first half) and B's head of step t
            # (second half). Entries: (chunk, stage, step_delta).
            #
            # Cross-step PSUM bank-zero safety (start=True wipes the bank):
            #  - ode1(t) zeroes A1: readers tanh_ode(t-1)/yode(t-1) are
            #    upstream of the (t-1) tail that gates ode1(t) via Sn.
            #  - g1x_u(t) zeroes A2: readers tanh_r/tanh_u(t-1) are
            #    upstream of rg2/ug2(t-1) -> ... -> ode1(t), and g1x_u(t)
            #    follows ode1(t) in the PE queue.
            #  - ns1x(t) zeroes B2: DVE queue order puts abs(t-1) and
            #    gtq_top(t-1) before add_top(t-1), which gates ode1(t).
            #  - rg2(t) zeroes B1: readers sig_r/sig_v(t-1) are upstream
            #    of ryc/gm(t-1) -> tail(t-1) -> ode1(t).
            WINDOW = [
                (0, s_ode1, 0),
                (0, s_g1x_u, 0),
                (0, s_g1x_r, 0),
                (1, s_rg2, -1),
                (0, s_tanh_ode, 0),
                (1, s_ug2, -1),
                (0, s_ns1x, 0),
                (1, s_sig_ur, -1),
                (1, s_ryc, -1),
                (0, s_ode2, 0),
                (1, s_gm, -1),
                (0, s_yode, 0),
                (1, s_ns1s, -1),
                (1, s_tanh_ns, -1),
                (0, s_g1s_r, 0),
                (0, s_g1s_u, 0),
                (1, s_uyneg, -1),
                (1, s_sneg, -1),
                (1, s_ns2, -1),
                (0, s_tanh_r, 0),
                (1, s_abs, -1),
                (1, s_gtq_top, -1),
                (1, s_add_top, -1),
                (0, s_tanh_u, 0),
                (1, s_gb, -1),
                (1, s_add_bot, -1),
                (1, s_ode1, 0),
                (1, s_g1x_u, 0),
                (0, s_rg2, 0),
                (1, s_g1x_r, 0),
                (1, s_tanh_ode, 0),
                (0, s_ug2, 0),
                (0, s_sig_ur, 0),
                (0, s_ryc, 0),
                (1, s_ns1x, 0),
                (0, s_gm, 0),
                (1, s_ode2, 0),
                (0, s_ns1s, 0),
                (1, s_yode, 0),
                (0, s_tanh_ns, 0),
                (1, s_g1s_r, 0),
                (1, s_g1s_u, 0),
                (0, s_uyneg, 0),
                (0, s_sneg, 0),
                (0, s_ns2, 0),
                (1, s_tanh_r, 0),
                (0, s_abs, 0),
                (0, s_gtq_top, 0),
                (0, s_add_top, 0),
                (1, s_tanh_u, 0),
                (0, s_gb, 0),
                (0, s_add_bot, 0),
            ]
            # epilogue: chunk B's tail of step T-1
            EPILOGUE = [(c, fn, dt_) for (c, fn, dt_) in WINDOW[:26]
                        if (c, dt_) == (1, -1)]

            xts = {}

            def fetch_x(t):
                xt = xpool.tile([INP, B], bf16, tag="xt", name=f"xt{t % 3}")
                nc.sync.dma_start(xt[:, :], xT_d[t])
                m2 = mpool.tile([INP, B], bf16, tag="m2", name=f"m2_{t % 3}")
                nc.sync.dma_start(m2[:, :], mT_d[t])
                xts[t] = (xt, m2)

            fetch_x(0)
            for t in range(T):
                if t + 1 < T:
                    fetch_x(t + 1)
                for c, fn, dlt in WINDOW:
                    tt = t + dlt
                    if tt < 0:
                        continue
                    fn(c, get_ctx(c, tt), *xts[tt])
                if t - 2 in xts:
                    xts.pop(t - 2)
            for c, fn, dlt in EPILOGUE:
                fn(c, get_ctx(c, T - 1), *xts[T - 1])

            # ---- final transform z0 = mlp2([y; s]) ----
            for c in range(CH):
                cs = slice(c * BC, (c + 1) * BC)
                Sf = S[c][T % 2]
                pt1 = bankA1[c][0:TZ, 0:BC]
                nc.tensor.matmul(pt1, wtz1[:, :], Sf[:, :],
                                 start=True, stop=True)
                h_t = tpool[c].tile([TZ, BC], bf16, tag="h_t")
                nc.scalar.activation(h_t[:, :], pt1, ACT.Tanh,
                                     bias=b_act(btz1) if use_bias else 0.0)
                pt2 = bankB1[c][0:2 * LAT, 0:BC]
                nc.tensor.matmul(pt2, wtz2[:, :], h_t[:, :],
                                 start=True, stop=True)
                zo = tpool[c].tile([2 * LAT, BC], f32, tag="zo")
                nc.scalar.activation(zo[0:LAT, :], pt2[0:LAT, :], ACT.Copy,
                                     bias=b_act(btz2t) if use_bias else 0.0)
                nc.scalar.activation(zo[LAT:2 * LAT, :], pt2[LAT:2 * LAT, :],
                                     ACT.Abs,
                                     bias=btz2b[LAT:2 * LAT, :] if use_bias else 0.0)
                nc.sync.dma_start(zout_d[:, cs], zo[:, :])

    try:
        import json as _json
        with open("/tmp/stage_map.json", "w") as _f:
            _json.dump(NAME2STAGE, _f)
    except Exception:
        pass
    nc.compile()
    return nc


def _prep(inputs):
    g = lambda k: np.ascontiguousarray(np.asarray(inputs[k], dtype=np.float32))
    data = g("data")
    tps = g("tps")
    W = {k: g(k) for k in (
        "ug_w1", "ug_b1", "ug_w2", "ug_b2", "rg_w1", "rg_b1", "rg_w2", "rg_b2",
        "ns_w1", "ns_b1", "ns_w2", "ns_b2", "ode_w1", "ode_b1", "ode_w2",
        "ode_b2", "tz_w1", "tz_b1", "tz_w2", "tz_b2")}

    rev = tps[::-1]
    dts = np.concatenate([np.full((1,), -0.01, np.float32),
                          rev[1:] - rev[:-1]]).astype(np.float32)
    dts = tuple(float(d) for d in dts.tolist())

    use_bias = any(float(np.abs(W[k]).max()) != 0.0 for k in W if "_b" in k)

    # time-reverse + transpose: [T, INP, N_TRAJ], contiguous
    xT_full = np.ascontiguousarray(data[:, ::-1, :].transpose(1, 2, 0))

    uniq = list(dict.fromkeys(dts))
    common = {
        "wode2dt": np.stack([np.float32(d) * W["ode_w2"] for d in uniq]),
        "wug1": W["ug_w1"],
        "wrg1": W["rg_w1"],
        "wns1": W["ns_w1"],
        "wug2nd": -np.concatenate([W["ug_w2"], W["ug_w2"]], axis=1),
        "wrg2d": np.concatenate([W["rg_w2"], W["rg_w2"]], axis=1),
        "wns2": W["ns_w2"],
        "wode1": W["ode_w1"],
        "wtz1": W["tz_w1"],
        "wtz2": W["tz_w2"],
        "zeros0": np.zeros((2 * LAT, B), np.float32),
    }
    f32_keys = set()
    if use_bias:
        col = lambda v: np.ascontiguousarray(v.reshape(-1, 1))
        row = lambda v: np.ascontiguousarray(v.reshape(1, -1))
        common.update({
            "bode1": col(W["ode_b1"]),
            "bug1": col(W["ug_b1"]),
            "brg1": col(W["rg_b1"]),
            "bns1": col(W["ns_b1"]),
            "bns2b": col(W["ns_b2"][LAT:]),
            "btz1": col(W["tz_b1"]),
            "btz2t": col(W["tz_b2"][:LAT]),
            "btz2b": col(W["tz_b2"][LAT:]),
            "bug2ndr": row(-np.concatenate([W["ug_b2"], W["ug_b2"]])),
            "brg2dr": row(np.concatenate([W["rg_b2"], W["rg_b2"]])),
            "bns2tr": row(W["ns_b2"][:LAT]),
            "bode2rdt": np.stack([np.float32(d) * W["ode_b2"].reshape(1, -1)
                                  for d in uniq]),
            "ones1": np.ones((1, BC), np.float32),
        })
        f32_keys = {"bode1", "bug1", "brg1", "bns1", "bns2b", "btz1",
                    "btz2t", "btz2b"}

    import ml_dtypes
    bf16 = ml_dtypes.bfloat16
    common = {k: np.ascontiguousarray(
                  v.astype(np.float32 if k in f32_keys else bf16))
              for k, v in common.items()}

    mask_rows = xT_full[:, NDATA:INP, :]
    mT_full = np.concatenate([mask_rows, mask_rows], axis=1)

    in_maps = []
    for c in range(NCORES):
        m = dict(common)
        m["xT"] = np.ascontiguousarray(
            xT_full[:, :, c * B:(c + 1) * B].astype(bf16))
        m["mT"] = np.ascontiguousarray(
            mT_full[:, :, c * B:(c + 1) * B].astype(bf16))
        in_maps.append(m)
    return in_maps, dts, use_bias


def _ensure_ntff_hook():
    """run_bass_kernel_spmd(trace=True) under axon imports
    antenv.axon_hooks, which is absent in this image. Install a stub so a
    BASS_TRACE=1 environment cannot crash the run."""
    import types as _types
    if "antenv.axon_hooks" in sys.modules:
        return
    hook = None
    try:
        from trn_agent_boot.trn_boot import _ntff_profile_via_ctypes
        hook = _ntff_profile_via_ctypes("/opt/axon/libaxon_pjrt.so")
    except Exception:
        hook = None
    try:
        import antenv
        mod = _types.ModuleType("antenv.axon_hooks")
        mod.get_axon_ntff_profile_hook = lambda: hook
        mod.set_axon_ntff_profile_hook = lambda h: None
        sys.modules["antenv.axon_hooks"] = mod
        antenv.axon_hooks = mod
    except Exception:
        pass


def _run(inputs, trace=False, trace_kwargs=None):
    _ensure_ntff_hook()
    from concourse.bass_utils import run_bass_kernel_spmd

    in_maps, dts, use_bias = _prep(inputs)
    key = (dts, use_bias)
    if key not in _cache:
        _cache[key] = _build(dts, use_bias)
    nc = _cache[key]

    res = run_bass_kernel_spmd(nc, in_maps, list(range(NCORES)),
                               trace=trace, **(trace_kwargs or {}))
    mu = np.empty((N_TRAJ, LAT), np.float32)
    std = np.empty((N_TRAJ, LAT), np.float32)
    for c in range(NCORES):
        z = res.results[c]["zout"]
        mu[c * B:(c + 1) * B] = z[0:LAT].T
        std[c * B:(c + 1) * B] = z[LAT:2 * LAT].T
    return (mu[None], std[None]), res


def kernel(**inputs):
    out, _ = _run(inputs, trace=False)
    return out
